# revision 2
# baseline (speedup 1.0000x reference)
"""TRN2 Bass kernel for nn_Attention_70257075028315.

reference:
    scores = einsum('bqd,bkd->bqk', query, key)       # B=8, Nq=Nk=2048, D=512
    probs  = softmax(scores, -1)
    out    = einsum('bqk,bkd->bqd', probs, key)

Sharding: batch b -> NeuronCore b (data parallel, fully local attention).

Per-core program (q/k: [2048, 512] fp32):
  Phase A/B: load K and Q in [128, 512] tiles; PE-transpose each into
    kT/qT [128(d), 4(dc), 16(kk/q tile), 128] stored as float32r (rounded by
    the PSUM->SBUF copy); K also cast to float32r natural layout for PV.
  Phase C (per q-tile, software-pipelined):
    S    = qT.T @ kT           accumulated over 4 d-chunks -> PSUM [128, 4, 512]
    max  per 512-chunk on DVE, combined and negated -> bias
    p    = exp(S - max) via one ACT pass PSUM->SBUF (f32r), fused row-sum
    pT   = PE-transpose of p (16x [128,128]) -> PSUM -> ACT copy to SBUF f32r
    o    = pT.T @ k_pv         accumulated over 16 kk-tiles -> PSUM [128, 512]
    out  = o * (1/rowsum)      on DVE, then DMA to DRAM.

float32r matmuls run at 1 cyc/row (vs 4 for fp32) with ~1.5e-2 max abs error
on N(0,512) scores (measured on HW) -> ~1e-3 relative error on the output.
"""

import numpy as np

import concourse.bass as bass
import concourse.tile as tile
import concourse.mybir as mybir
from concourse import bacc
from concourse.bass_utils import run_bass_kernel_spmd
from concourse.masks import make_identity

FP32 = mybir.dt.float32
FP32R = mybir.dt.float32r
AF = mybir.ActivationFunctionType

B, NQ, NK, D = 8, 2048, 2048, 512
P = 128
NKT = NK // P   # 16 kk tiles
NQT = NQ // P   # 16 q tiles
NDC = D // P    # 4 d chunks
NCH = NK // 512  # 4 score chunks of 512


def build(score_dtype=FP32R, repeat_c=1):
    nc = bacc.Bacc("TRN2", target_bir_lowering=False, debug=False)
    q_d = nc.dram_tensor("query", [NQ, D], FP32, kind="ExternalInput").ap()
    k_d = nc.dram_tensor("key", [NK, D], FP32, kind="ExternalInput").ap()
    out_d = nc.dram_tensor("out", [NQ, D], FP32, kind="ExternalOutput").ap()

    q_tiles_d = q_d.rearrange("(t p) d -> t p d", p=P)
    k_tiles_d = k_d.rearrange("(t p) d -> t p d", p=P)
    out_tiles_d = out_d.rearrange("(t p) d -> t p d", p=P)

    with tile.TileContext(nc) as tc:
        _body(tc, q_tiles_d, k_tiles_d, out_tiles_d, score_dtype, repeat_c)
    nc.compile()
    return nc


def _body(tc, q_tiles_d, k_tiles_d, out_tiles_d, score_dtype, repeat_c):
    from contextlib import ExitStack

    nc = tc.nc
    with ExitStack() as ctx:
        persist = ctx.enter_context(tc.tile_pool(name="persist", bufs=1))
        work = ctx.enter_context(tc.tile_pool(name="work", bufs=2))
        small = ctx.enter_context(tc.tile_pool(name="small", bufs=3))
        ps_s = ctx.enter_context(tc.tile_pool(name="ps_s", bufs=1, space="PSUM"))
        ps_tr = ctx.enter_context(tc.tile_pool(name="ps_tr", bufs=2, space="PSUM"))
        ps_pv = ctx.enter_context(tc.tile_pool(name="ps_pv", bufs=2, space="PSUM"))

        ident = persist.tile([P, P], FP32)
        make_identity(nc, ident[:])
        ident_r = persist.tile([P, P], FP32R)
        nc.vector.tensor_copy(ident_r[:], ident[:])

        # Transposed operands: [d%128, d-chunk, kk-tile, 128]
        kT = persist.tile([P, NDC, NKT, P], score_dtype)
        qT = persist.tile([P, NDC, NQT, P], score_dtype)
        k_pv = persist.tile([P, NKT, 512 // P, P], FP32R)  # natural [kk, d]

        # ---- Phase A/B: load + transpose ----
        with tc.tile_pool(name="load", bufs=6) as load:
            for src_d, dstT, pv in ((k_tiles_d, kT, k_pv), (q_tiles_d, qT, None)):
                for g in range(4):  # groups of 4 tiles
                    tiles = []
                    for j in range(4):
                        t = load.tile([P, D], FP32, tag="ld")
                        nc.sync.dma_start(t[:], src_d[g * 4 + j])
                        tiles.append(t)
                    if pv is not None:
                        for j in range(4):
                            nc.vector.tensor_copy(
                                pv[:, g * 4 + j], tiles[j][:].rearrange("p (a b) -> p a b", b=P)
                            )
                    for dc in range(NDC):
                        ptr = ps_tr.tile([P, 4, P], FP32, tag="tr")
                        for j in range(4):
                            nc.tensor.transpose(
                                ptr[:, j, :],
                                tiles[j][:, dc * P : (dc + 1) * P],
                                ident[:],
                            )
                        eng = nc.scalar if dstT is kT else nc.vector
                        if dstT is kT:
                            nc.scalar.copy(dstT[:, dc, g * 4 : (g + 1) * 4, :], ptr[:])
                        else:
                            nc.vector.tensor_copy(
                                dstT[:, dc, g * 4 : (g + 1) * 4, :], ptr[:]
                            )

        # ---- Phase C: attention over q tiles, software-pipelined ----
        def emit_S(i):
            """S matmuls + chunk maxes + negmax for q-tile i."""
            psum_s = ps_s.tile([P, NCH, 512], FP32, tag="s")
            m4 = small.tile([P, NCH], FP32, tag="m4")
            negmax = small.tile([P, 1], FP32, tag="negmax")
            for c in range(NCH):
                for dc in range(NDC):
                    nc.tensor.matmul(
                        psum_s[:, c, :],
                        lhsT=qT[:, dc, i, :],
                        rhs=kT[:, dc, c * 4 : (c + 1) * 4, :],
                        start=(dc == 0),
                        stop=(dc == NDC - 1),
                    )
                nc.vector.reduce_max(
                    m4[:, c : c + 1], psum_s[:, c, :], axis=mybir.AxisListType.X
                )
            nc.vector.reduce_max(
                negmax[:], m4[:], axis=mybir.AxisListType.X, negate=True
            )
            return psum_s, negmax

        def emit_E(i, psum_s, negmax):
            """exp(S - max) -> p (f32r) + fused row-sum; reciprocal."""
            p = work.tile([P, NCH, 512], FP32R, tag="p")
            rowsum = small.tile([P, 1], FP32, tag="rowsum")
            rinv = small.tile([P, 1], FP32, tag="rinv")
            nc.scalar.activation(
                p[:], psum_s[:], AF.Exp, bias=negmax[:], accum_out=rowsum[:]
            )
            nc.vector.reciprocal(rinv[:], rowsum[:])
            return p, rinv

        def emit_T(i, p):
            """Transpose p -> pT [128(kk), 16 tiles, 128(q)] f32r."""
            pT = work.tile([P, NKT, P], FP32R, tag="pT")
            for g in range(4):
                ptr = ps_tr.tile([P, 4, P], FP32R, tag="tr")
                for j in range(4):
                    nc.tensor.transpose(
                        ptr[:, j, :],
                        p[:, g, j * P : (j + 1) * P],
                        ident_r[:],
                    )
                nc.scalar.copy(pT[:, g * 4 : (g + 1) * 4, :], ptr[:])
            return pT

        def emit_PV(i, pT, rinv):
            psum_o = ps_pv.tile([P, 512], FP32, tag="pv")
            for t in range(NKT):
                nc.tensor.matmul(
                    psum_o[:],
                    lhsT=pT[:, t, :],
                    rhs=k_pv[:, t],
                    start=(t == 0),
                    stop=(t == NKT - 1),
                )
            out_sb = work.tile([P, 512], FP32, tag="out_sb")
            nc.vector.tensor_scalar_mul(out_sb[:], psum_o[:], rinv[:])
            nc.sync.dma_start(out_tiles_d[i], out_sb[:])

        for _ in range(repeat_c):
            state = {}
            s0 = emit_S(0)
            state[0] = (*s0, *emit_E(0, *s0))
            for i in range(NQT):
                psum_s, negmax, p, rinv = state.pop(i)
                pT = emit_T(i, p)
                if i + 1 < NQT:
                    s1 = emit_S(i + 1)
                    state[i + 1] = (*s1, *emit_E(i + 1, *s1))
                emit_PV(i, pT, rinv)


_NC_CACHE = {}


def _get_nc(score_dtype=FP32R, repeat_c=1):
    key = (str(score_dtype), repeat_c)
    if key not in _NC_CACHE:
        _NC_CACHE[key] = build(score_dtype, repeat_c)
    return _NC_CACHE[key]


def kernel(query: np.ndarray, key: np.ndarray) -> np.ndarray:
    query = np.asarray(query, dtype=np.float32)
    key = np.asarray(key, dtype=np.float32)
    assert query.shape == (B, NQ, D) and key.shape == (B, NK, D)
    nc = _get_nc()
    in_maps = [{"query": query[b], "key": key[b]} for b in range(B)]
    res = run_bass_kernel_spmd(nc, in_maps, list(range(B)))
    return np.stack([res.results[b]["out"] for b in range(B)], axis=0)
